# revision 2
# baseline (speedup 1.0000x reference)
"""Trainium2 Bass kernel v3 for nn_NeuralTrustNetwork (gnn_message_passing).

out[e] = lrelu(lrelu(c) @ W_mlp + b_mlp) @ wL + bL
         + (x[src]*x[dst]) @ w1 + b1 + (w[src]*w[dst]) @ w2 + b2
  with c = (s1+s2)[src] + (p1+p2)[dst]

v3 = v2 with ZERO DVE instructions in the steady state.  Any DVE op on
16-bit data runs in 2-port perf mode and takes the SBUF shared port pair,
fully blocking GPSIMD - which is busy generating SWDGE gather descriptors
the whole kernel.  Measured: v2 'full' 611ms vs 'gather' 11ms per pass.

All elementwise work is re-expressed on PE + ACT (dedicated SBUF ports):
- c = s+p: accumulate the two pair-transposes into one PSUM tile
  (is_transpose matmuls with start/stop accumulation), lrelu on ACT.
- u = <v_src, v_dst~> (v = [w|x], 128-dim): square trick
  u = (0.5(v_s+v~_d))^2 - (0.5(v_s-v~_d))^2 summed; the +/- combos are
  I/-I matmul accumulations into PSUM, squares+row-sums are ACT
  Square with accum_out, and the final sum lands in e1 via I/-I matmuls.
- final + (bL+b1+b2): ACT Copy bias.
"""

from contextlib import ExitStack

import numpy as np

import concourse.bacc as bacc
import concourse.mybir as mybir
import concourse.tile as tile
from concourse.masks import make_identity

FP16 = mybir.dt.float16
F32 = mybir.dt.float32
I16 = mybir.dt.int16

NCORES = 8
T = 1024           # edges per tile (one gather pair)
D = 64

_prog_cache = {}

# gather position i <- edge stream position q(i) = (i%128)*8 + i//128
_I = np.arange(T)
_QPERM = (_I % 128) * (T // 128) + _I // 128


def _wrap_idx(idx):
    """[1024] int -> [128, 64] int16 (wrap 16 partitions, replicate 8x)."""
    w = idx.reshape(-1, 16).T.astype(np.int16)  # [16, 64]
    return np.tile(w, (8, 1))


def _build_program(nb, nhalf, reps=1, variant="full",
                   act=mybir.ActivationFunctionType.Lrelu,
                   plan_a=True):
    """nb[k] = tiles for bucket k (src-half*2 + dst-half); nhalf = rows/half."""
    tot = sum(nb)
    nc = bacc.Bacc(
        "TRN2",
        target_bir_lowering=False,
        debug=False,
        enable_asserts=False,
        num_swdge_queues=1,
    )
    src_lo = nc.dram_tensor("src_lo", [nhalf, 256], FP16, kind="ExternalInput").ap()
    src_hi = nc.dram_tensor("src_hi", [nhalf, 256], FP16, kind="ExternalInput").ap()
    dst_lo = nc.dram_tensor("dst_lo", [nhalf, 256], FP16, kind="ExternalInput").ap()
    dst_hi = nc.dram_tensor("dst_hi", [nhalf, 256], FP16, kind="ExternalInput").ap()
    idx_d = nc.dram_tensor("idx", [128, tot * 128], I16, kind="ExternalInput").ap()
    wbd_d = nc.dram_tensor("wbd", [128, 128], FP16, kind="ExternalInput").ap()
    bm2_d = nc.dram_tensor("bm2", [128, 1], F32, kind="ExternalInput").ap()
    wlp_d = nc.dram_tensor("wlp", [128, 2], FP16, kind="ExternalInput").ap()
    ineg_d = nc.dram_tensor("ineg", [128, 128], FP16, kind="ExternalInput").ap()
    out_d = nc.dram_tensor("out", [tot * 128, 8], F32, kind="ExternalOutput").ap()

    s_tabs = [src_lo, src_lo, src_hi, src_hi]
    d_tabs = [dst_lo, dst_hi, dst_lo, dst_hi]

    kb_bias = None  # set via closure in run(); baked as float into ACT

    with tile.TileContext(nc) as tc, ExitStack() as ctx:
        const = ctx.enter_context(tc.tile_pool(name="const", bufs=1))
        idx_t = const.tile([128, tot * 128], I16)
        nc.sync.dma_start(idx_t[:], idx_d[:])
        ident = const.tile([128, 128], FP16)
        make_identity(nc, ident[:])
        ineg_t = const.tile([128, 128], FP16)
        nc.sync.dma_start(ineg_t[:], ineg_d[:])
        idf_t = const.tile([128, 128], F32)
        make_identity(nc, idf_t[:])
        inegf_d = nc.dram_tensor("inegf", [128, 128], F32,
                                 kind="ExternalInput").ap()
        inegf_t = const.tile([128, 128], F32)
        nc.sync.dma_start(inegf_t[:], inegf_d[:])
        wbd_t = const.tile([128, 128], FP16)
        nc.sync.dma_start(wbd_t[:], wbd_d[:])
        bm2_t = const.tile([128, 1], F32)
        nc.sync.dma_start(bm2_t[:], bm2_d[:])
        wlp_t = const.tile([128, 2], FP16)
        nc.sync.dma_start(wlp_t[:], wlp_d[:])
        kb_d = nc.dram_tensor("kb", [128, 1], F32, kind="ExternalInput").ap()
        kb_t = const.tile([128, 1], F32)
        nc.sync.dma_start(kb_t[:], kb_d[:])

        gp = ctx.enter_context(tc.tile_pool(name="gath", bufs=3))
        atp = ctx.enter_context(tc.tile_pool(name="at", bufs=4))
        l2p = ctx.enter_context(tc.tile_pool(name="l2", bufs=4))
        sqp = ctx.enter_context(tc.tile_pool(name="sq", bufs=2))
        uap = ctx.enter_context(tc.tile_pool(name="uacc", bufs=2))
        lcp = ctx.enter_context(tc.tile_pool(name="lc", bufs=2))
        op = ctx.enter_context(tc.tile_pool(name="outs", bufs=3))
        ps_t = ctx.enter_context(tc.tile_pool(name="ps_t", bufs=2, space="PSUM"))
        ps_h = ctx.enter_context(tc.tile_pool(name="ps_h", bufs=2, space="PSUM"))
        ps_e = ctx.enter_context(tc.tile_pool(name="ps_e", bufs=1, space="PSUM"))
        ps_u = ctx.enter_context(tc.tile_pool(name="ps_u", bufs=1, space="PSUM"))
        ps_c = ctx.enter_context(tc.tile_pool(name="ps_c", bufs=1, space="PSUM"))

        for rep in range(reps):
            t = 0
            for k in range(4):
                for _ in range(nb[k]):
                    S = gp.tile([128, 8, 256], FP16, tag="S")
                    Dt = gp.tile([128, 8, 256], FP16, tag="D")
                    if variant != "compute":
                        nc.gpsimd.dma_gather(
                            out_ap=S[:], in_ap=s_tabs[k][:],
                            idxs_ap=idx_t[:, t * 128:t * 128 + 64],
                            num_idxs=T, num_idxs_reg=T, elem_size=256,
                            queue_num=0,
                        )
                        nc.gpsimd.dma_gather(
                            out_ap=Dt[:], in_ap=d_tabs[k][:],
                            idxs_ap=idx_t[:, t * 128 + 64:t * 128 + 128],
                            num_idxs=T, num_idxs_reg=T, elem_size=256,
                            queue_num=0,
                        )
                    if variant == "gather":
                        t += 1
                        continue

                    # contiguous staging copies (ACT; PE matmul APs must
                    # be contiguous, gather tile slices are strided)
                    cs = lcp.tile([128, 8, 64], FP16, tag="cs")
                    nc.scalar.copy(cs[:], S[:, :, 0:64])
                    cd = lcp.tile([128, 8, 64], FP16, tag="cd")
                    nc.scalar.copy(cd[:], Dt[:, :, 0:64])
                    sv = lcp.tile([128, 8, 128], FP16, tag="sv")
                    nc.scalar.copy(sv[:], S[:, :, 64:192])
                    dv = lcp.tile([128, 8, 128], FP16, tag="dv")
                    nc.scalar.copy(dv[:], Dt[:, :, 64:192])
                    # c = s+p via I-matmul accumulation (f32 PSUM), lrelu
                    pc8 = ps_c.tile([128, 8, 64], F32, tag="pc8")
                    nc.tensor.matmul(pc8[:], lhsT=ident[:], rhs=cs[:],
                                     start=True, stop=False)
                    nc.tensor.matmul(pc8[:], lhsT=ident[:], rhs=cd[:],
                                     start=False, stop=True)
                    lc = lcp.tile([128, 8, 64], FP16, tag="lc")
                    nc.scalar.activation(lc[:], pc8[:], act, alpha=0.01)

                    # u-head: plus/minus accumulations [128, 4, 128] f32
                    ua = uap.tile([128, 8], F32, tag="up")
                    um = uap.tile([128, 8], F32, tag="um")
                    sq = sqp.tile([128, 128], F32, tag="sq")
                    for hh in range(2):
                        pp = ps_u.tile([128, 4, 128], F32, tag="pp")
                        nc.tensor.matmul(pp[:], lhsT=ident[:],
                                         rhs=sv[:, 4 * hh:4 * hh + 4, :],
                                         start=True, stop=False)
                        nc.tensor.matmul(pp[:], lhsT=ident[:],
                                         rhs=dv[:, 4 * hh:4 * hh + 4, :],
                                         start=False, stop=True)
                        pm = ps_u.tile([128, 4, 128], F32, tag="pm")
                        nc.tensor.matmul(pm[:], lhsT=ident[:],
                                         rhs=sv[:, 4 * hh:4 * hh + 4, :],
                                         start=True, stop=False)
                        nc.tensor.matmul(pm[:], lhsT=ineg_t[:],
                                         rhs=dv[:, 4 * hh:4 * hh + 4, :],
                                         start=False, stop=True)
                        for j in range(4):
                            sl = 4 * hh + j
                            nc.scalar.activation(
                                sq[:], pp[:, j, :],
                                mybir.ActivationFunctionType.Square,
                                scale=0.5, accum_out=ua[:, sl:sl + 1])
                            nc.scalar.activation(
                                sq[:], pm[:, j, :],
                                mybir.ActivationFunctionType.Square,
                                scale=0.5, accum_out=um[:, sl:sl + 1])

                    # MLP per slot-pair: transpose 2 slots -> [128, 128]
                    e1 = ps_e.tile([128, 8], F32, tag="e1")
                    for u2 in range(4):
                        pc = ps_t.tile([128, 128], FP16, tag="pc")
                        nc.tensor.matmul(pc[:],
                                         lhsT=lc[:, 2 * u2:2 * u2 + 2, :],
                                         rhs=ident[:], is_transpose=True,
                                         start=True, stop=True)
                        at = atp.tile([128, 128], FP16, tag="at")
                        nc.scalar.copy(at[:], pc[:])
                        ph = ps_h.tile([128, 128], F32, tag="ph")
                        nc.tensor.matmul(ph[:], lhsT=wbd_t[:], rhs=at[:],
                                         start=True, stop=True)
                        l2 = l2p.tile([128, 128], FP16, tag="l2")
                        nc.scalar.activation(l2[:], ph[:], act,
                                             bias=bm2_t[:, 0:1], alpha=0.01)
                        nc.tensor.matmul(e1[:, 2 * u2:2 * u2 + 2], lhsT=l2[:],
                                         rhs=wlp_t[:], start=True, stop=False)
                        # fold u in: e1 += ua - um (ACT scale 0.5 made 1/4)
                        nc.tensor.matmul(e1[:, 2 * u2:2 * u2 + 2],
                                         lhsT=idf_t[:],
                                         rhs=ua[:, 2 * u2:2 * u2 + 2],
                                         start=False, stop=False)
                        nc.tensor.matmul(e1[:, 2 * u2:2 * u2 + 2],
                                         lhsT=inegf_t[:],
                                         rhs=um[:, 2 * u2:2 * u2 + 2],
                                         start=False, stop=True)

                    ot = op.tile([128, 8], F32, tag="ot")
                    nc.scalar.activation(ot[:], e1[:],
                                         mybir.ActivationFunctionType.Identity,
                                         bias=kb_t[:, 0:1])
                    nc.sync.dma_start(out_d[t * 128:(t + 1) * 128, :], ot[:])
                    t += 1

    nc.compile()
    return nc


def _prep(inputs):
    src = np.asarray(inputs["src"]).astype(np.int64).ravel()
    dst = np.asarray(inputs["dst"]).astype(np.int64).ravel()
    s = (np.asarray(inputs["s1"], np.float32)
         + np.asarray(inputs["s2"], np.float32))
    p = (np.asarray(inputs["p1"], np.float32)
         + np.asarray(inputs["p2"], np.float32))
    x = np.asarray(inputs["x"], np.float32)
    w = np.asarray(inputs["w"], np.float32)
    w1 = np.asarray(inputs["w1"], np.float32).ravel()
    w2 = np.asarray(inputs["w2"], np.float32).ravel()

    E = src.shape[0]
    N = s.shape[0]
    assert E % NCORES == 0
    epc = E // NCORES
    nhalf = (N + 1) // 2

    z = np.zeros_like(x)
    src_tab = np.concatenate([s, w, x, z], axis=1).astype(np.float16)
    dst_tab = np.concatenate([p, w * w2[None, :], x * w1[None, :], z],
                             axis=1).astype(np.float16)
    if N < 2 * nhalf:
        pad = np.zeros((2 * nhalf - N, 256), np.float16)
        src_tab = np.vstack([src_tab, pad])
        dst_tab = np.vstack([dst_tab, pad])

    per_core = []
    counts = np.zeros((NCORES, 4), np.int64)
    for c in range(NCORES):
        sc = src[c * epc:(c + 1) * epc]
        dc = dst[c * epc:(c + 1) * epc]
        b = (sc >= nhalf) * 2 + (dc >= nhalf)
        ords = [np.flatnonzero(b == k) for k in range(4)]
        counts[c] = [len(o) for o in ords]
        per_core.append((sc, dc, ords))

    nb = [int(-(-counts[:, k].max() // T)) for k in range(4)]
    tot = sum(nb)

    idx_all = np.zeros((NCORES, 128, tot * 128), np.int16)
    order_all = np.full((NCORES, tot * T), -1, np.int64)

    for c in range(NCORES):
        sc, dc, ords = per_core[c]
        t = 0
        pos = 0
        for k in range(4):
            ids = ords[k]
            cap = nb[k] * T
            se = np.zeros(cap, np.int64)
            de = np.zeros(cap, np.int64)
            se[:len(ids)] = sc[ids] - (nhalf if k >= 2 else 0)
            de[:len(ids)] = dc[ids] - (nhalf if k % 2 == 1 else 0)
            order_all[c, pos:pos + len(ids)] = ids
            pos += cap
            for bi in range(nb[k]):
                idx_all[c, :, t * 128:t * 128 + 64] = _wrap_idx(
                    se[bi * T + _QPERM])
                idx_all[c, :, t * 128 + 64:t * 128 + 128] = _wrap_idx(
                    de[bi * T + _QPERM])
                t += 1

    W_mlp = np.asarray(inputs["W_mlp"], np.float32)
    b_mlp = np.asarray(inputs["b_mlp"], np.float32).ravel()
    wL = np.asarray(inputs["wL"], np.float32).ravel()
    kb = (float(np.asarray(inputs["bL"]).ravel()[0])
          + float(np.asarray(inputs["b1"]).ravel()[0])
          + float(np.asarray(inputs["b2"]).ravel()[0]))

    wbd = np.zeros((128, 128), np.float16)
    wbd[:64, :64] = W_mlp.astype(np.float16)
    wbd[64:, 64:] = W_mlp.astype(np.float16)
    wlp = np.zeros((128, 2), np.float16)
    wlp[:64, 0] = wL.astype(np.float16)
    wlp[64:, 1] = wL.astype(np.float16)
    bm2 = np.concatenate([b_mlp, b_mlp]).astype(np.float32).reshape(128, 1)
    ineg = (-np.eye(128)).astype(np.float16)

    weights = dict(
        wbd=wbd, bm2=bm2, wlp=wlp, ineg=ineg,
        inegf=(-np.eye(128)).astype(np.float32),
        kb=np.full((128, 1), kb, np.float32),
    )
    tabs = dict(
        src_lo=np.ascontiguousarray(src_tab[:nhalf]),
        src_hi=np.ascontiguousarray(src_tab[nhalf:]),
        dst_lo=np.ascontiguousarray(dst_tab[:nhalf]),
        dst_hi=np.ascontiguousarray(dst_tab[nhalf:]),
    )
    return tuple(nb), nhalf, epc, E, tabs, weights, idx_all, order_all


def run(inputs, **spmd_kwargs):
    from concourse.bass_utils import run_bass_kernel_spmd

    nb, nhalf, epc, E, tabs, weights, idx_all, order_all = _prep(inputs)

    key = (nb, nhalf)
    if key not in _prog_cache:
        _prog_cache[key] = _build_program(list(nb), nhalf)
    nc = _prog_cache[key]

    in_maps = []
    for c in range(NCORES):
        m = dict(tabs)
        m.update(weights)
        m["idx"] = idx_all[c]
        in_maps.append(m)

    res = run_bass_kernel_spmd(nc, in_maps, list(range(NCORES)), **spmd_kwargs)

    out = np.empty((E, 1), np.float32)
    for c in range(NCORES):
        oc = np.asarray(res.results[c]["out"], np.float32).reshape(-1)
        order = order_all[c]
        valid = order >= 0
        out[c * epc + order[valid], 0] = oc[valid]
    return out, res


def kernel(**inputs) -> np.ndarray:
    out, _ = run(inputs)
    return out


# revision 3
# speedup vs baseline: 2.9925x; 2.9925x over previous
"""Trainium2 Bass kernel for nn_NeuralTrustNetwork (gnn_message_passing).

out[e] = lrelu(lrelu(c) @ W_mlp + b_mlp) @ wL + bL
         + (x[src]*x[dst]) @ w1 + b1 + (w[src]*w[dst]) @ w2 + b2
  with c = (s1+s2)[src] + (p1+p2)[dst]

Strategy (edge-parallel across 8 NeuronCores, per the sharding hint):
- Host: build combined fp16 node tables SRC=[s1|s2|x|w], DST=[p1|p2|x|w]
  (512B rows), split into lo/hi halves (dma_gather indices are int16),
  bucket each core's edges by (src-half, dst-half), pad buckets to
  2048-edge batches.
- Device per 2048-edge batch: 4x dma_gather (1024 rows each — SWDGE
  descriptor-ring limit), DVE adds for c, PE pair-transposes + blockdiag
  W_mlp matmul for the MLP, fused LeakyReLU+bias on ACT, dot-product
  heads on DVE, everything accumulated into a [16,128] PSUM tile that
  stores contiguously.
"""

from contextlib import ExitStack

import numpy as np

import concourse.bacc as bacc
import concourse.bass as bass
import concourse.mybir as mybir
import concourse.tile as tile
from concourse.masks import make_identity

FP16 = mybir.dt.float16
F32 = mybir.dt.float32
I16 = mybir.dt.int16

NCORES = 8
B = 2048          # edges per compute batch
BG = 1024         # edges per dma_gather (descriptor ring limit)
J = B // 128      # 16 slots
D = 64

_prog_cache = {}

# gather position i -> DRAM/out position q: q = (i%128)*16 + 8*(i//1024) + (i//128)%8
_I = np.arange(B)
_QPERM = (_I % 128) * (B // 128) + (B // 1024) * 4 * (_I // 1024) + (_I // 128) % 8
# inverse: gather list position i must hold stream edge (base + _QPERM[i])


def _wrap_idx_chunk(idx):
    """[1024] int -> [128, 64] int16 (wrap 16 partitions, replicate 8x)."""
    w = idx.reshape(-1, 16).T.astype(np.int16)  # [16, 64]
    return np.tile(w, (8, 1))


def _build_program(nb, nhalf, reps=1, variant='full'):
    """Build + compile the SPMD program for per-bucket batch counts nb[0..3]."""
    totb = sum(nb)
    nc = bacc.Bacc(
        "TRN2",
        target_bir_lowering=False,
        debug=False,
        enable_asserts=False,
        num_swdge_queues=4,
    )
    src_lo = nc.dram_tensor("src_lo", [nhalf, 256], FP16, kind="ExternalInput").ap()
    src_hi = nc.dram_tensor("src_hi", [nhalf, 256], FP16, kind="ExternalInput").ap()
    dst_lo = nc.dram_tensor("dst_lo", [nhalf, 256], FP16, kind="ExternalInput").ap()
    dst_hi = nc.dram_tensor("dst_hi", [nhalf, 256], FP16, kind="ExternalInput").ap()
    idx_s = nc.dram_tensor("idx_s", [totb * 128, 128], I16, kind="ExternalInput").ap()
    idx_d = nc.dram_tensor("idx_d", [totb * 128, 128], I16, kind="ExternalInput").ap()
    wbd_d = nc.dram_tensor("wbd", [128, 128], FP16, kind="ExternalInput").ap()
    wlp_d = nc.dram_tensor("wlp", [128, 2], FP16, kind="ExternalInput").ap()
    bm2_d = nc.dram_tensor("bm2", [128, 1], F32, kind="ExternalInput").ap()
    w12_d = nc.dram_tensor("w12", [1, 2048], FP16, kind="ExternalInput").ap()
    b3_d = nc.dram_tensor("b3", [1, 3], F32, kind="ExternalInput").ap()
    out_d = nc.dram_tensor("out", [totb * 128, 16], F32, kind="ExternalOutput").ap()

    s_tabs = [src_lo, src_lo, src_hi, src_hi]
    d_tabs = [dst_lo, dst_hi, dst_lo, dst_hi]

    with tile.TileContext(nc) as tc, ExitStack() as ctx:
        const = ctx.enter_context(tc.tile_pool(name="const", bufs=1))
        ident = const.tile([128, 128], FP16)
        make_identity(nc, ident[:])
        wbd_t = const.tile([128, 128], FP16)
        nc.sync.dma_start(wbd_t[:], wbd_d[:])
        wlp_t = const.tile([128, 2], FP16)
        nc.sync.dma_start(wlp_t[:], wlp_d[:])
        bm2_t = const.tile([128, 1], F32)
        nc.sync.dma_start(bm2_t[:], bm2_d[:])
        w12r_t = const.tile([1, 2048], FP16)
        nc.sync.dma_start(w12r_t[:], w12_d[:])
        b3_t = const.tile([1, 3], F32)
        nc.sync.dma_start(b3_t[:], b3_d[:])
        ones1 = const.tile([1, 128], FP16)
        nc.vector.memset(ones1[:], 1.0)
        ones128f = const.tile([1, 128], F32)
        nc.vector.memset(ones128f[:], 1.0)

        w12big = const.tile([128, 2048], FP16)
        k128 = const.tile([128, 1], F32)
        with tc.tile_pool(name="setup_ps", bufs=1, space="PSUM") as sps:
            for q in range(4):
                pw = sps.tile([128, 512], F32)
                nc.tensor.matmul(
                    pw[:], lhsT=ones1[:], rhs=w12r_t[:, q * 512:(q + 1) * 512],
                    start=True, stop=True,
                )
                nc.scalar.copy(w12big[:, q * 512:(q + 1) * 512], pw[:])
            pk = sps.tile([128, 3], F32)
            nc.tensor.matmul(pk[:], lhsT=ones128f[:], rhs=b3_t[:], start=True, stop=True)
            nc.vector.tensor_reduce(k128[:], pk[:], axis=mybir.AxisListType.X,
                                    op=mybir.AluOpType.add)

        idxp = ctx.enter_context(tc.tile_pool(name="idx", bufs=3))
        gp = ctx.enter_context(tc.tile_pool(name="gath", bufs=3))
        cp = ctx.enter_context(tc.tile_pool(name="csum", bufs=2))
        atp = ctx.enter_context(tc.tile_pool(name="at", bufs=4))
        l2p = ctx.enter_context(tc.tile_pool(name="l2", bufs=4))
        mp = ctx.enter_context(tc.tile_pool(name="m2", bufs=2))
        outp = ctx.enter_context(tc.tile_pool(name="outs", bufs=3))
        ps_c = ctx.enter_context(tc.tile_pool(name="ps_c", bufs=2, space="PSUM"))
        ps_h = ctx.enter_context(tc.tile_pool(name="ps_h", bufs=2, space="PSUM"))
        ps_o = ctx.enter_context(tc.tile_pool(name="ps_o", bufs=2, space="PSUM"))

        for rep in range(reps):
          t = 0
          for k in range(4):
            s_tab, d_tab = s_tabs[k], d_tabs[k]
            for _ in range(nb[k]):
                si = idxp.tile([128, 128], I16, tag="si")
                nc.sync.dma_start(si[:], idx_s[t * 128:(t + 1) * 128, :])
                di = idxp.tile([128, 128], I16, tag="di")
                nc.sync.dma_start(di[:], idx_d[t * 128:(t + 1) * 128, :])

                S = gp.tile([128, J, 256], FP16, tag="S")
                Dt = gp.tile([128, J, 256], FP16, tag="D")
                for h in range(2 if variant != 'compute' else 0):
                    nc.gpsimd.dma_gather(
                        out_ap=S[:, h * 8:(h + 1) * 8, :], in_ap=s_tab[:],
                        idxs_ap=si[:, h * 64:(h + 1) * 64],
                        num_idxs=BG, num_idxs_reg=BG, elem_size=256,
                        queue_num=2 * h,
                    )
                    nc.gpsimd.dma_gather(
                        out_ap=Dt[:, h * 8:(h + 1) * 8, :], in_ap=d_tab[:],
                        idxs_ap=di[:, h * 64:(h + 1) * 64],
                        num_idxs=BG, num_idxs_reg=BG, elem_size=256,
                        queue_num=2 * h + 1,
                    )

                if variant == 'gather':
                    t += 1
                    continue
                # c = (s1+s2)[src] + (p1+p2)[dst]   [128, J, 64] fp16
                t1 = cp.tile([128, J, D], FP16, tag="t1")
                nc.vector.tensor_tensor(t1[:], S[:, :, 0:64], S[:, :, 64:128],
                                        op=mybir.AluOpType.add)
                t2 = cp.tile([128, J, D], FP16, tag="t2")
                nc.vector.tensor_tensor(t2[:], Dt[:, :, 0:64], Dt[:, :, 64:128],
                                        op=mybir.AluOpType.add)
                c = cp.tile([128, J, D], FP16, tag="c")
                nc.vector.tensor_tensor(c[:], t1[:], t2[:], op=mybir.AluOpType.add)

                # dot-product heads: m2 = sum_d x_s*x_d*w1 + w_s*w_d*w2  [128, J]
                tmp = mp.tile([128, J, 128], FP16, tag="tmp")
                nc.vector.tensor_tensor(tmp[:], S[:, :, 128:256],
                                        w12big[:].rearrange("p (j e) -> p j e", j=J),
                                        op=mybir.AluOpType.mult)
                tmp2 = mp.tile([128, J, 128], FP16, tag="tmp2")
                nc.vector.tensor_tensor(tmp2[:], tmp[:], Dt[:, :, 128:256],
                                        op=mybir.AluOpType.mult)
                m2 = mp.tile([128, J], F32, tag="m2")
                nc.vector.tensor_reduce(m2[:], tmp2[:], axis=mybir.AxisListType.X,
                                        op=mybir.AluOpType.add)
                # MLP path per slot-pair; e1 accumulates edge-major [128, J]
                e1 = ps_o.tile([128, J], F32)
                for u in range(8):
                    pc = ps_c.tile([128, 128], FP16, tag="pc")
                    nc.tensor.matmul(pc[:], lhsT=c[:, 2 * u:2 * u + 2, :],
                                     rhs=ident[:], is_transpose=True,
                                     start=True, stop=True)
                    at = atp.tile([128, 128], FP16, tag="at")
                    nc.scalar.activation(at[:], pc[:],
                                         mybir.ActivationFunctionType.Lrelu,
                                         alpha=0.01)
                    ph = ps_h.tile([128, 128], F32, tag="ph")
                    nc.tensor.matmul(ph[:], lhsT=wbd_t[:], rhs=at[:],
                                     start=True, stop=True)
                    l2 = l2p.tile([128, 128], FP16, tag="l2")
                    nc.scalar.activation(l2[:], ph[:],
                                         mybir.ActivationFunctionType.Lrelu,
                                         bias=bm2_t[:, 0:1], alpha=0.01)
                    nc.tensor.matmul(e1[:, 2 * u:2 * u + 2], lhsT=l2[:],
                                     rhs=wlp_t[:], start=True, stop=True)

                ot = outp.tile([128, J], F32)
                nc.vector.tensor_tensor(ot[:], e1[:], m2[:], op=mybir.AluOpType.add)
                nc.vector.tensor_scalar_add(ot[:], ot[:], k128[:, 0:1])
                nc.sync.dma_start(out_d[t * 128:(t + 1) * 128, :], ot[:])
                t += 1

    nc.compile()
    return nc


def _prep(inputs):
    src = np.asarray(inputs["src"]).astype(np.int64).ravel()
    dst = np.asarray(inputs["dst"]).astype(np.int64).ravel()
    s1 = np.asarray(inputs["s1"], np.float32)
    s2 = np.asarray(inputs["s2"], np.float32)
    p1 = np.asarray(inputs["p1"], np.float32)
    p2 = np.asarray(inputs["p2"], np.float32)
    x = np.asarray(inputs["x"], np.float32)
    w = np.asarray(inputs["w"], np.float32)

    E = src.shape[0]
    N = s1.shape[0]
    assert E % NCORES == 0
    epc = E // NCORES
    nhalf = (N + 1) // 2

    src_tab = np.concatenate([s1, s2, x, w], axis=1).astype(np.float16)
    dst_tab = np.concatenate([p1, p2, x, w], axis=1).astype(np.float16)
    if N < 2 * nhalf:
        padrow = np.zeros((2 * nhalf - N, 256), np.float16)
        src_tab = np.vstack([src_tab, padrow])
        dst_tab = np.vstack([dst_tab, padrow])

    # bucket per core
    per_core = []
    counts = np.zeros((NCORES, 4), np.int64)
    for c in range(NCORES):
        s = src[c * epc:(c + 1) * epc]
        d = dst[c * epc:(c + 1) * epc]
        b = (s >= nhalf) * 2 + (d >= nhalf)
        ords = [np.flatnonzero(b == k) for k in range(4)]
        counts[c] = [len(o) for o in ords]
        per_core.append((s, d, ords))

    nb = [int(-(-counts[:, k].max() // B)) for k in range(4)]
    totb = sum(nb)

    idx_s_all = np.zeros((NCORES, totb * 128, 128), np.int16)
    idx_d_all = np.zeros((NCORES, totb * 128, 128), np.int16)
    order_all = np.full((NCORES, totb * B), -1, np.int64)

    for c in range(NCORES):
        s, d, ords = per_core[c]
        t = 0
        pos = 0
        for k in range(4):
            ids = ords[k]
            cap = nb[k] * B
            se = np.zeros(cap, np.int64)
            de = np.zeros(cap, np.int64)
            se[:len(ids)] = s[ids] - (nhalf if k >= 2 else 0)
            de[:len(ids)] = d[ids] - (nhalf if k % 2 == 1 else 0)
            order_all[c, pos:pos + len(ids)] = ids
            pos += cap
            for bi in range(nb[k]):
                blk_s = np.empty((128, 128), np.int16)
                blk_d = np.empty((128, 128), np.int16)
                seg_s = se[bi * B + _QPERM]
                seg_d = de[bi * B + _QPERM]
                for h in range(2):
                    sl = slice(h * BG, (h + 1) * BG)
                    blk_s[:, h * 64:(h + 1) * 64] = _wrap_idx_chunk(seg_s[sl])
                    blk_d[:, h * 64:(h + 1) * 64] = _wrap_idx_chunk(seg_d[sl])
                idx_s_all[c, t * 128:(t + 1) * 128] = blk_s
                idx_d_all[c, t * 128:(t + 1) * 128] = blk_d
                t += 1

    # weights
    W_mlp = np.asarray(inputs["W_mlp"], np.float32)
    b_mlp = np.asarray(inputs["b_mlp"], np.float32).ravel()
    wL = np.asarray(inputs["wL"], np.float32).ravel()
    w1 = np.asarray(inputs["w1"], np.float32).ravel()
    w2 = np.asarray(inputs["w2"], np.float32).ravel()
    bL = float(np.asarray(inputs["bL"]).ravel()[0])
    b1 = float(np.asarray(inputs["b1"]).ravel()[0])
    b2 = float(np.asarray(inputs["b2"]).ravel()[0])

    wbd = np.zeros((128, 128), np.float16)
    wbd[:64, :64] = W_mlp.astype(np.float16)
    wbd[64:, 64:] = W_mlp.astype(np.float16)
    wlp = np.zeros((128, 2), np.float16)
    wlp[:64, 0] = wL.astype(np.float16)
    wlp[64:, 1] = wL.astype(np.float16)
    bm2 = np.concatenate([b_mlp, b_mlp]).astype(np.float32).reshape(128, 1)
    w12 = np.tile(np.concatenate([w1, w2]).astype(np.float16), J).reshape(1, 2048)
    b3 = np.array([[bL, b1, b2]], np.float32)

    weights = dict(wbd=wbd, wlp=wlp, bm2=bm2, w12=w12, b3=b3)
    tabs = dict(
        src_lo=np.ascontiguousarray(src_tab[:nhalf]),
        src_hi=np.ascontiguousarray(src_tab[nhalf:]),
        dst_lo=np.ascontiguousarray(dst_tab[:nhalf]),
        dst_hi=np.ascontiguousarray(dst_tab[nhalf:]),
    )
    return (tuple(nb), nhalf, epc, E, tabs, weights,
            idx_s_all, idx_d_all, order_all)


def run(inputs, **spmd_kwargs):
    """Returns (output [E,1] float32, BassKernelResults)."""
    from concourse.bass_utils import run_bass_kernel_spmd

    (nb, nhalf, epc, E, tabs, weights,
     idx_s_all, idx_d_all, order_all) = _prep(inputs)

    key = (nb, nhalf)
    if key not in _prog_cache:
        _prog_cache[key] = _build_program(list(nb), nhalf)
    nc = _prog_cache[key]

    in_maps = []
    for c in range(NCORES):
        m = dict(tabs)
        m.update(weights)
        m["idx_s"] = idx_s_all[c]
        m["idx_d"] = idx_d_all[c]
        in_maps.append(m)

    res = run_bass_kernel_spmd(nc, in_maps, list(range(NCORES)), **spmd_kwargs)

    out = np.empty((E, 1), np.float32)
    for c in range(NCORES):
        oc = np.asarray(res.results[c]["out"], np.float32).reshape(-1)
        order = order_all[c]
        valid = order >= 0
        out[c * epc + order[valid], 0] = oc[valid]
    return out, res


def kernel(**inputs) -> np.ndarray:
    out, _ = run(inputs)
    return out


# revision 4
# speedup vs baseline: 4.5660x; 1.5258x over previous
"""Trainium2 Bass kernel v4: instruction-lean baseline restructure.

Same math as the baseline; wall time on this rig scales with engine
instruction count (~40-250us/inst), so v4 cuts per-2048-edge-batch
instructions from ~55 to ~31:
- host precomputes s=s1+s2, p=p1+p2 and folds w1/w2 into the dst table
  (tables [s|w|x|0] / [p|w*w2|x*w1|0]): c-add is 1 DVE op, heads are
  mult+reduce (2 ops)  [6 -> 3]
- all gather indices loaded in ONE upfront DMA  [-2 DMA/batch]
- the 8 pair-transposes write one PSUM tile -> ONE lrelu, TWO wide W_mlp
  matmuls, TWO l2 activations  [24 -> 13]
- final + const via ACT Identity bias (no DVE scalar-add)
Gathers all on SWDGE queue 0 (multi-queue round-robin measured 5x slower).
"""

from contextlib import ExitStack

import numpy as np

import concourse.bacc as bacc
import concourse.mybir as mybir
import concourse.tile as tile
from concourse.masks import make_identity

FP16 = mybir.dt.float16
F32 = mybir.dt.float32
I16 = mybir.dt.int16

NCORES = 8
B = 2048          # edges per compute batch
BG = 1024         # edges per dma_gather (descriptor ring limit)
J = B // 128      # 16 slots
D = 64

_prog_cache = {}

# gather position i -> out stream position q
_I = np.arange(B)
_QPERM = (_I % 128) * (B // 128) + (B // 1024) * 4 * (_I // 1024) + (_I // 128) % 8


def _wrap_idx_chunk(idx):
    """[1024] int -> [128, 64] int16 (wrap 16 partitions, replicate 8x)."""
    w = idx.reshape(-1, 16).T.astype(np.int16)  # [16, 64]
    return np.tile(w, (8, 1))


def _build_program(nb, nhalf, reps=1, variant="full",
                   act=mybir.ActivationFunctionType.Lrelu):
    totb = sum(nb)
    nc = bacc.Bacc(
        "TRN2",
        target_bir_lowering=False,
        debug=False,
        enable_asserts=False,
        num_swdge_queues=1,
    )
    src_lo = nc.dram_tensor("src_lo", [nhalf, 256], FP16, kind="ExternalInput").ap()
    src_hi = nc.dram_tensor("src_hi", [nhalf, 256], FP16, kind="ExternalInput").ap()
    dst_lo = nc.dram_tensor("dst_lo", [nhalf, 256], FP16, kind="ExternalInput").ap()
    dst_hi = nc.dram_tensor("dst_hi", [nhalf, 256], FP16, kind="ExternalInput").ap()
    # per batch: 128 cols src idx (2 gathers) + 128 cols dst idx
    idx_d = nc.dram_tensor("idx", [128, totb * 256], I16, kind="ExternalInput").ap()
    wbd_d = nc.dram_tensor("wbd", [128, 128], FP16, kind="ExternalInput").ap()
    bm2_d = nc.dram_tensor("bm2", [128, 1], F32, kind="ExternalInput").ap()
    wlp_d = nc.dram_tensor("wlp", [128, 2], FP16, kind="ExternalInput").ap()
    kb_d = nc.dram_tensor("kb", [128, 1], F32, kind="ExternalInput").ap()
    out_d = nc.dram_tensor("out", [totb * 128, 16], F32, kind="ExternalOutput").ap()

    s_tabs = [src_lo, src_lo, src_hi, src_hi]
    d_tabs = [dst_lo, dst_hi, dst_lo, dst_hi]

    with tile.TileContext(nc) as tc, ExitStack() as ctx:
        const = ctx.enter_context(tc.tile_pool(name="const", bufs=1))
        idx_t = const.tile([128, totb * 256], I16)
        nc.sync.dma_start(idx_t[:], idx_d[:])
        ident = const.tile([128, 128], FP16)
        make_identity(nc, ident[:])
        wbd_t = const.tile([128, 128], FP16)
        nc.sync.dma_start(wbd_t[:], wbd_d[:])
        bm2_t = const.tile([128, 1], F32)
        nc.sync.dma_start(bm2_t[:], bm2_d[:])
        wlp_t = const.tile([128, 2], FP16)
        nc.sync.dma_start(wlp_t[:], wlp_d[:])
        kb_t = const.tile([128, 1], F32)
        nc.sync.dma_start(kb_t[:], kb_d[:])

        gp = ctx.enter_context(tc.tile_pool(name="gath", bufs=3))
        cp = ctx.enter_context(tc.tile_pool(name="csum", bufs=2))
        up = ctx.enter_context(tc.tile_pool(name="umul", bufs=2))
        urp = ctx.enter_context(tc.tile_pool(name="ured", bufs=2))
        atp = ctx.enter_context(tc.tile_pool(name="at", bufs=2))
        l2p = ctx.enter_context(tc.tile_pool(name="l2", bufs=2))
        outp = ctx.enter_context(tc.tile_pool(name="outs", bufs=3))
        ps_t = ctx.enter_context(tc.tile_pool(name="ps_t", bufs=2, space="PSUM"))
        ps_h = ctx.enter_context(tc.tile_pool(name="ps_h", bufs=2, space="PSUM"))
        ps_e = ctx.enter_context(tc.tile_pool(name="ps_e", bufs=2, space="PSUM"))

        for rep in range(reps):
            t = 0
            for k in range(4):
                for _ in range(nb[k]):
                    S = gp.tile([128, J, 256], FP16, tag="S")
                    Dt = gp.tile([128, J, 256], FP16, tag="D")
                    ib = t * 256
                    for h in range(2 if variant != "compute" else 0):
                        nc.gpsimd.dma_gather(
                            out_ap=S[:, h * 8:(h + 1) * 8, :], in_ap=s_tabs[k][:],
                            idxs_ap=idx_t[:, ib + h * 64:ib + (h + 1) * 64],
                            num_idxs=BG, num_idxs_reg=BG, elem_size=256,
                            queue_num=0,
                        )
                        nc.gpsimd.dma_gather(
                            out_ap=Dt[:, h * 8:(h + 1) * 8, :], in_ap=d_tabs[k][:],
                            idxs_ap=idx_t[:, ib + 128 + h * 64:ib + 128 + (h + 1) * 64],
                            num_idxs=BG, num_idxs_reg=BG, elem_size=256,
                            queue_num=0,
                        )
                    if variant == "gather":
                        t += 1
                        continue

                    # c = s[src] + p[dst]   [128, J, 64]
                    c = cp.tile([128, J, D], FP16, tag="c")
                    nc.vector.tensor_tensor(c[:], S[:, :, 0:64], Dt[:, :, 0:64],
                                            op=mybir.AluOpType.add)
                    # heads: u = sum([w|x]_src * [w~|x~]_dst)  [128, J]
                    um = up.tile([128, J, 128], FP16, tag="um")
                    nc.vector.tensor_tensor(um[:], S[:, :, 64:192],
                                            Dt[:, :, 64:192],
                                            op=mybir.AluOpType.mult)
                    ur = urp.tile([128, J], F32, tag="ur")
                    nc.vector.tensor_reduce(ur[:], um[:],
                                            axis=mybir.AxisListType.X,
                                            op=mybir.AluOpType.add)

                    # 8 pair-transposes into ONE psum tile, one lrelu,
                    # two wide W matmuls, two l2 activations
                    pca = ps_t.tile([128, 8, 128], FP16, tag="pca")
                    for u in range(8):
                        nc.tensor.matmul(pca[:, u, :],
                                         lhsT=c[:, 2 * u:2 * u + 2, :],
                                         rhs=ident[:], is_transpose=True,
                                         start=True, stop=True)
                    ata = atp.tile([128, 8, 128], FP16, tag="ata")
                    nc.scalar.activation(ata[:], pca[:], act, alpha=0.01)
                    l2a = l2p.tile([128, 8, 128], FP16, tag="l2a")
                    for hh in range(2):
                        ph = ps_h.tile([128, 4, 128], F32, tag="ph")
                        nc.tensor.matmul(ph[:], lhsT=wbd_t[:],
                                         rhs=ata[:, 4 * hh:4 * hh + 4, :],
                                         start=True, stop=True)
                        nc.scalar.activation(l2a[:, 4 * hh:4 * hh + 4, :], ph[:],
                                             act, bias=bm2_t[:, 0:1], alpha=0.01)
                    e1 = ps_e.tile([128, J], F32, tag="e1")
                    for u in range(8):
                        nc.tensor.matmul(e1[:, 2 * u:2 * u + 2],
                                         lhsT=l2a[:, u, :], rhs=wlp_t[:],
                                         start=True, stop=True)

                    os1 = outp.tile([128, J], F32, tag="os1")
                    nc.vector.tensor_tensor(os1[:], e1[:], ur[:],
                                            op=mybir.AluOpType.add)
                    ot = outp.tile([128, J], F32, tag="ot")
                    nc.scalar.activation(ot[:], os1[:],
                                         mybir.ActivationFunctionType.Identity,
                                         bias=kb_t[:, 0:1])
                    nc.sync.dma_start(out_d[t * 128:(t + 1) * 128, :], ot[:])
                    t += 1

    nc.compile()
    return nc


def _prep(inputs):
    src = np.asarray(inputs["src"]).astype(np.int64).ravel()
    dst = np.asarray(inputs["dst"]).astype(np.int64).ravel()
    s = (np.asarray(inputs["s1"], np.float32)
         + np.asarray(inputs["s2"], np.float32))
    p = (np.asarray(inputs["p1"], np.float32)
         + np.asarray(inputs["p2"], np.float32))
    x = np.asarray(inputs["x"], np.float32)
    w = np.asarray(inputs["w"], np.float32)
    w1 = np.asarray(inputs["w1"], np.float32).ravel()
    w2 = np.asarray(inputs["w2"], np.float32).ravel()

    E = src.shape[0]
    N = s.shape[0]
    assert E % NCORES == 0
    epc = E // NCORES
    nhalf = (N + 1) // 2

    z = np.zeros_like(x)
    src_tab = np.concatenate([s, w, x, z], axis=1).astype(np.float16)
    dst_tab = np.concatenate([p, w * w2[None, :], x * w1[None, :], z],
                             axis=1).astype(np.float16)
    if N < 2 * nhalf:
        pad = np.zeros((2 * nhalf - N, 256), np.float16)
        src_tab = np.vstack([src_tab, pad])
        dst_tab = np.vstack([dst_tab, pad])

    per_core = []
    counts = np.zeros((NCORES, 4), np.int64)
    for c in range(NCORES):
        sc = src[c * epc:(c + 1) * epc]
        dc = dst[c * epc:(c + 1) * epc]
        b = (sc >= nhalf) * 2 + (dc >= nhalf)
        ords = [np.flatnonzero(b == k) for k in range(4)]
        counts[c] = [len(o) for o in ords]
        per_core.append((sc, dc, ords))

    nb = [int(-(-counts[:, k].max() // B)) for k in range(4)]
    totb = sum(nb)

    idx_all = np.zeros((NCORES, 128, totb * 256), np.int16)
    order_all = np.full((NCORES, totb * B), -1, np.int64)

    for c in range(NCORES):
        sc, dc, ords = per_core[c]
        t = 0
        pos = 0
        for k in range(4):
            ids = ords[k]
            cap = nb[k] * B
            se = np.zeros(cap, np.int64)
            de = np.zeros(cap, np.int64)
            se[:len(ids)] = sc[ids] - (nhalf if k >= 2 else 0)
            de[:len(ids)] = dc[ids] - (nhalf if k % 2 == 1 else 0)
            order_all[c, pos:pos + len(ids)] = ids
            pos += cap
            for bi in range(nb[k]):
                seg_s = se[bi * B + _QPERM]
                seg_d = de[bi * B + _QPERM]
                ib = t * 256
                for h in range(2):
                    sl = slice(h * BG, (h + 1) * BG)
                    idx_all[c, :, ib + h * 64:ib + (h + 1) * 64] = (
                        _wrap_idx_chunk(seg_s[sl]))
                    idx_all[c, :, ib + 128 + h * 64:ib + 128 + (h + 1) * 64] = (
                        _wrap_idx_chunk(seg_d[sl]))
                t += 1

    W_mlp = np.asarray(inputs["W_mlp"], np.float32)
    b_mlp = np.asarray(inputs["b_mlp"], np.float32).ravel()
    wL = np.asarray(inputs["wL"], np.float32).ravel()
    kb = (float(np.asarray(inputs["bL"]).ravel()[0])
          + float(np.asarray(inputs["b1"]).ravel()[0])
          + float(np.asarray(inputs["b2"]).ravel()[0]))

    wbd = np.zeros((128, 128), np.float16)
    wbd[:64, :64] = W_mlp.astype(np.float16)
    wbd[64:, 64:] = W_mlp.astype(np.float16)
    wlp = np.zeros((128, 2), np.float16)
    wlp[:64, 0] = wL.astype(np.float16)
    wlp[64:, 1] = wL.astype(np.float16)
    bm2 = np.concatenate([b_mlp, b_mlp]).astype(np.float32).reshape(128, 1)

    weights = dict(
        wbd=wbd, bm2=bm2, wlp=wlp,
        kb=np.full((128, 1), kb, np.float32),
    )
    tabs = dict(
        src_lo=np.ascontiguousarray(src_tab[:nhalf]),
        src_hi=np.ascontiguousarray(src_tab[nhalf:]),
        dst_lo=np.ascontiguousarray(dst_tab[:nhalf]),
        dst_hi=np.ascontiguousarray(dst_tab[nhalf:]),
    )
    return tuple(nb), nhalf, epc, E, tabs, weights, idx_all, order_all


def run(inputs, **spmd_kwargs):
    from concourse.bass_utils import run_bass_kernel_spmd

    nb, nhalf, epc, E, tabs, weights, idx_all, order_all = _prep(inputs)

    key = (nb, nhalf)
    if key not in _prog_cache:
        _prog_cache[key] = _build_program(list(nb), nhalf)
    nc = _prog_cache[key]

    in_maps = []
    for c in range(NCORES):
        m = dict(tabs)
        m.update(weights)
        m["idx"] = idx_all[c]
        in_maps.append(m)

    res = run_bass_kernel_spmd(nc, in_maps, list(range(NCORES)), **spmd_kwargs)

    out = np.empty((E, 1), np.float32)
    for c in range(NCORES):
        oc = np.asarray(res.results[c]["out"], np.float32).reshape(-1)
        order = order_all[c]
        valid = order >= 0
        out[c * epc + order[valid], 0] = oc[valid]
    return out, res


def kernel(**inputs) -> np.ndarray:
    out, _ = run(inputs)
    return out
